# revision 12
# baseline (speedup 1.0000x reference)
"""Trainium2 Bass kernel for nn_LocalFWL (GNN link prediction, LocalFWL-style).

Strategy:
 - Host does integer-only index preprocessing: edge-multiplicity matrix with
   self-loops folded in (cnt + I, exact in bf16), degree counts, last-write-wins
   dedup of duplicate edges, and per-query path-pair enumeration (the sparse
   structure of the D1@D2 contraction). Floating-point math runs on device.
 - 8-core SPMD, queries (pos pairs) sharded by core; GCN replicated:
   * GCN via dense normalized-adjacency matmuls in bf16 (counts are small
     integers - exact; features lose <0.5% - well within tolerance).
   * Per-edge MLPs only on the <=1024 edges each core actually needs.
   * All row gathers via the Q7 dma_gather instruction (one instruction per
     1024-row gather) out of permuted DRAM tables laid out so table writes are
     2KB-contiguous per partition.
"""
import sys

sys.path.insert(0, "/opt/pypackages")
sys.path.insert(0, "/opt/trn_rl_repo")

import numpy as np
import ml_dtypes

N, E, P, IN, H = 1024, 65536, 1024, 128, 64
NCORES = 8
QPC = P // NCORES
LN_EPS = 1e-5
NB = N // 128  # 8 node blocks

BF16 = ml_dtypes.bfloat16


def _ceil_to(x, m):
    return ((x + m - 1) // m) * m


def _wrap16(vals):
    """int16 index layout for dma_gather: idx t at [t%16, t//16], replicated
    to 128 partitions (8 groups of 16)."""
    ni = len(vals)
    assert ni % 16 == 0
    base = np.asarray(vals, np.int16).reshape(ni // 16, 16).T  # [16, ni//16]
    return np.ascontiguousarray(np.tile(base, (8, 1)))  # [128, ni//16]


def preprocess(ei, pos):
    """All-integer index preprocessing."""
    ei0 = np.asarray(ei[0], np.int64)
    ei1 = np.asarray(ei[1], np.int64)
    p0 = np.asarray(pos[0], np.int64)
    p1 = np.asarray(pos[1], np.int64)

    flat = ei0 * N + ei1
    cnt = np.bincount(flat, minlength=N * N).reshape(N, N)  # [r, c] multiplicity
    cnt[np.arange(N), np.arange(N)] += 1  # fold self-loop into the adjacency
    deg = np.bincount(ei1, minlength=N) + 1

    last = np.full(N * N, -1, np.int64)
    last[flat] = np.arange(E)  # last occurrence wins (matches jnp .at[].set)
    PA = last.reshape(N, N)

    per_core = []
    maxM, maxNE = 0, 0
    rowoks = PA >= 0
    for c in range(NCORES):
        qs = slice(c * QPC, (c + 1) * QPC)
        i_q, j_q = p0[qs], p1[qs]
        pairs = []
        allids = []
        for q in range(QPC):
            ks = np.nonzero(rowoks[i_q[q]] & rowoks[:, j_q[q]])[0]
            a, b = PA[i_q[q], ks], PA[ks, j_q[q]]
            pairs.append((a, b))
            allids.append(a)
            allids.append(b)
        maxM = max(maxM, max(len(a) for a, _ in pairs))
        needed = np.unique(np.concatenate(allids)) if any(len(a) for a in allids) \
            else np.zeros(1, np.int64)
        per_core.append((i_q, j_q, pairs, needed))
        maxNE = max(maxNE, len(needed))

    M = max(4, maxM)
    ne_pad = max(128, _ceil_to(maxNE, 128))
    NE_J = ne_pad // 128

    # permuted row id of node n in h_tbl (so h_tbl writes are 2KB-contiguous)
    permh = lambda n: (n % 128) * NB + n // 128
    # permuted row id of needed-edge slot t in x1/x2_loc
    permx = lambda t: (t % 128) * NE_J + t // 128
    ZROW = 128 * NE_J  # zeroed pad row in x_loc tables

    NI1 = ne_pad + 128
    NI2 = M * 128

    cores = []
    for c in range(NCORES):
        i_q, j_q, pairs, needed = per_core[c]
        ne = len(needed)
        loc = {int(g): t for t, g in enumerate(needed)}
        # edge-endpoint gather lists (edges t<ne_pad, then the 128 pos rows)
        v0 = np.zeros(NI1, np.int64)
        v1 = np.zeros(NI1, np.int64)
        for t in range(ne_pad):
            g = needed[t] if t < ne else needed[0]
            v0[t] = permh(ei0[g])
            v1[t] = permh(ei1[g])
        for q in range(QPC):
            v0[ne_pad + q] = permh(i_q[q])
            v1[ne_pad + q] = permh(j_q[q])
        # pair gather lists: entry (q, m) at t = m*128 + q
        w1 = np.full(NI2, ZROW, np.int64)
        w2 = np.full(NI2, ZROW, np.int64)
        for q in range(QPC):
            a, b = pairs[q]
            for m in range(len(a)):
                w1[m * 128 + q] = permx(loc[int(a[m])])
                w2[m * 128 + q] = permx(loc[int(b[m])])
        cores.append(dict(
            idxg0=_wrap16(v0), idxg1=_wrap16(v1),
            idxp1=_wrap16(w1), idxp2=_wrap16(w2),
        ))

    # cnt in bf16, cb-major: cntb[p, cb*N + rb*128 + cc] = cnt[rb*128+p, cb*128+cc]
    cntb = np.ascontiguousarray(
        cnt.reshape(NB, 128, NB, 128).transpose(1, 2, 0, 3).reshape(128, NB * N)
        .astype(BF16))
    degf = np.ascontiguousarray(deg.reshape(NB, 128).T.astype(np.float32))
    shared = dict(cntb=cntb, degf=degf, M=M, NE_J=NE_J, ne_pad=ne_pad)
    return shared, cores


_PROGRAM_CACHE = {}


def build_program(M, NE_J, stages=6):
    import concourse.bacc as bacc
    import concourse.bass as bass
    import concourse.tile as tile
    import concourse.mybir as mybir
    from concourse.masks import make_identity
    from concourse import library_config

    dt = mybir.dt
    f32, bf16, i16 = dt.float32, dt.bfloat16, dt.int16
    AF = mybir.ActivationFunctionType
    OP = mybir.AluOpType
    ne_pad = NE_J * 128
    NI1 = ne_pad + 128
    NI2 = M * 128
    NJ1 = NI1 // 128  # he gather blocks (NE_J edge blocks + 1 pos block)
    nc = bacc.Bacc("TRN2", target_bir_lowering=False, debug=False)

    def din(name, shape, d=f32):
        return nc.dram_tensor(name, shape, d, kind="ExternalInput").ap()

    xTb = din("xTb", [IN, N], bf16)
    cntb = din("cntb", [128, NB * N], bf16)
    degf = din("degf", [128, NB])
    Wg1 = din("Wg1", [IN, H], bf16)
    Wg2 = din("Wg2", [H, H], bf16)
    Wm1e = din("Wm1e", [H + 1, H], bf16)
    Wm2e = din("Wm2e", [H + 1, H], bf16)
    Wa = din("Wa", [2 * H, H], bf16)
    Wb = din("Wb", [H, 1], bf16)
    bg1r = din("bg1r", [128, H]); bg2r = din("bg2r", [128, H])
    gm1r = din("gm1r", [128, H]); bem1r = din("bem1r", [128, H])
    gm2r = din("gm2r", [128, H]); bem2r = din("bem2r", [128, H])
    bar = din("bar", [128, H]); bbr = din("bbr", [128, 1])
    idxg0 = din("idxg0", [128, NI1 // 16], i16)
    idxg1 = din("idxg1", [128, NI1 // 16], i16)
    idxp1 = din("idxp1", [128, NI2 // 16], i16)
    idxp2 = din("idxp2", [128, NI2 // 16], i16)

    h_tbl = nc.dram_tensor("h_tbl", [N, H], f32).ap()
    x1_loc = nc.dram_tensor("x1_loc", [ne_pad + 8, H], f32).ap()
    x2_loc = nc.dram_tensor("x2_loc", [ne_pad + 8, H], f32).ap()
    outq = nc.dram_tensor("outq", [128, 1], f32, kind="ExternalOutput").ap()

    with tile.TileContext(nc) as tc:
        with tc.tile_pool(name="const", bufs=1) as cp, \
             tc.tile_pool(name="work", bufs=2) as wp, \
             tc.tile_pool(name="psP", bufs=2, space="PSUM") as pp:

            nc.gpsimd.load_library(library_config.mlp)

            identb = cp.tile([128, 128], bf16)
            make_identity(nc, identb[:])
            identf = cp.tile([128, 128], f32)
            make_identity(nc, identf[:])

            def load(ap_dram, shape, d=f32):
                t = cp.tile(shape, d, tag=ap_dram.name)
                nc.sync.dma_start(t[:], ap_dram[:])
                return t

            xTb_s = load(xTb, [IN, N], bf16)
            Wg1_s = load(Wg1, [IN, H], bf16)
            Wg2_s = load(Wg2, [H, H], bf16)
            Wm1_s = load(Wm1e, [H + 1, H], bf16)
            Wm2_s = load(Wm2e, [H + 1, H], bf16)
            Wa_s = load(Wa, [2 * H, H], bf16)
            Wb_s = load(Wb, [H, 1], bf16)
            bg1_s = load(bg1r, [128, H]); bg2_s = load(bg2r, [128, H])
            gm1_s = load(gm1r, [128, H]); bem1_s = load(bem1r, [128, H])
            gm2_s = load(gm2r, [128, H]); bem2_s = load(bem2r, [128, H])
            ba_s = load(bar, [128, H]); bb_s = load(bbr, [128, 1])
            degf_s = load(degf, [128, NB])

            dinvp = cp.tile([128, NB], f32)
            nc.scalar.activation(dinvp[:], degf_s[:], AF.Sqrt)
            nc.vector.reciprocal(dinvp[:], dinvp[:])

            # cnt cb-major slices (issue DMAs early; the cb accumulation
            # chain starts as soon as its own slice lands)
            cnt_s = cp.tile([128, NB * N], bf16)
            for cb in range(NB):
                nc.sync.dma_start(cnt_s[:, cb * N:(cb + 1) * N],
                                  cntb[:, cb * N:(cb + 1) * N])

            # ---- GCN layer 1 ----
            xw1s = cp.tile([128, NB, H], bf16)
            for rb in range(NB):
                ps = pp.tile([128, H], f32, tag="ps")
                nc.tensor.matmul(ps[:], lhsT=xTb_s[:, rb * 128:(rb + 1) * 128],
                                 rhs=Wg1_s[:], start=True, stop=True)
                nc.vector.tensor_scalar_mul(xw1s[:, rb, :], ps[:],
                                            dinvp[:, rb:rb + 1])
            h1b = cp.tile([128, NB, H], bf16)
            for cb in range(NB):
                agg = pp.tile([128, H], f32, tag="agg")
                for rb in range(NB):
                    nc.tensor.matmul(
                        agg[:],
                        lhsT=cnt_s[:, cb * N + rb * 128: cb * N + (rb + 1) * 128],
                        rhs=xw1s[:, rb, :], start=(rb == 0), stop=(rb == NB - 1))
                nc.vector.scalar_tensor_tensor(
                    out=h1b[:, cb, :], in0=agg[:], scalar=dinvp[:, cb:cb + 1],
                    in1=bg1_s[:], op0=OP.mult, op1=OP.add)

            if stages < 2:
                dmy = wp.tile([128, 1], f32, tag="dmy")
                nc.vector.tensor_copy(out=dmy[:], in_=h1b[:, 0, 0:1])
                nc.sync.dma_start(outq[:], dmy[:])

            if stages >= 2:
                # ---- GCN layer 2 ----
                h1T_s = cp.tile([H, NB * 128], bf16)
                for cb in range(NB):
                    pt = pp.tile([H, 128], bf16, tag="tpb")
                    nc.tensor.transpose(pt[:], h1b[:, cb, :], identb[:])
                    nc.scalar.activation(h1T_s[:, cb * 128:(cb + 1) * 128],
                                         pt[:], AF.Copy)
                xw2s = cp.tile([128, NB, H], bf16)
                for rb in range(NB):
                    ps = pp.tile([128, H], f32, tag="ps")
                    nc.tensor.matmul(ps[:], lhsT=h1T_s[:, rb * 128:(rb + 1) * 128],
                                     rhs=Wg2_s[:], start=True, stop=True)
                    nc.vector.tensor_scalar_mul(xw2s[:, rb, :], ps[:],
                                                dinvp[:, rb:rb + 1])
                h2f = cp.tile([128, NB, H], f32)
                for cb in range(NB):
                    agg = pp.tile([128, H], f32, tag="agg")
                    for rb in range(NB):
                        nc.tensor.matmul(
                            agg[:],
                            lhsT=cnt_s[:, cb * N + rb * 128: cb * N + (rb + 1) * 128],
                            rhs=xw2s[:, rb, :], start=(rb == 0), stop=(rb == NB - 1))
                    nc.vector.scalar_tensor_tensor(
                        out=h2f[:, cb, :], in0=agg[:], scalar=dinvp[:, cb:cb + 1],
                        in1=bg2_s[:], op0=OP.mult, op1=OP.add)
                # h_tbl row (p*NB + cb) = node cb*128+p; 2KB/partition write
                nc.sync.dma_start(
                    h_tbl[:].rearrange("(p b) h -> p (b h)", p=128),
                    h2f[:, :, :].rearrange("p b h -> p (b h)"))

            if stages == 2:
                dmy = wp.tile([128, 1], f32, tag="dmy")
                nc.vector.tensor_copy(out=dmy[:], in_=h2f[:, 0, 0:1])
                nc.sync.dma_start(outq[:], dmy[:])

            def gather_rows(dst, src_dram, idx_tile, ni):
                # HW limit: <=1024 indices per dma_gather call
                j0 = 0
                while j0 * 128 < ni:
                    nj = min(8, ni // 128 - j0)
                    nc.gpsimd.dma_gather(
                        dst[:, j0:j0 + nj, :], src_dram,
                        idx_tile[:, j0 * 8:(j0 + nj) * 8],
                        nj * 128, nj * 128, H)
                    j0 += nj

            if stages >= 3:
                # ---- gather h rows for needed edges + pos queries ----
                ie0 = cp.tile([128, NI1 // 16], i16)
                ie1 = cp.tile([128, NI1 // 16], i16)
                nc.sync.dma_start(ie0[:], idxg0[:])
                nc.sync.dma_start(ie1[:], idxg1[:])
                he0 = cp.tile([128, NJ1, H], f32)
                he1 = cp.tile([128, NJ1, H], f32)
                gather_rows(he0, h_tbl[:], ie0, NI1)
                gather_rows(he1, h_tbl[:], ie1, NI1)
                xeb = cp.tile([128, NE_J, H], bf16)
                nc.vector.tensor_tensor(out=xeb[:, :, :],
                                        in0=he0[:, 0:NE_J, :],
                                        in1=he1[:, 0:NE_J, :], op=OP.mult)

            if stages == 3:
                dmy = wp.tile([128, 1], f32, tag="dmy")
                nc.vector.tensor_copy(out=dmy[:], in_=he0[:, 0, 0:1])
                nc.sync.dma_start(outq[:], dmy[:])

            if stages >= 4:
                # ---- edge MLPs (Linear+bias folded, LN, relu) ----
                xeT_s = cp.tile([H + 1, NE_J * 128], bf16)
                nc.vector.memset(xeT_s[H:H + 1, :], 1.0)
                for j in range(NE_J):
                    pt = pp.tile([H, 128], bf16, tag="tpb")
                    nc.tensor.transpose(pt[:], xeb[:, j, :], identb[:])
                    nc.scalar.activation(xeT_s[0:H, j * 128:(j + 1) * 128],
                                         pt[:], AF.Copy)
                x1pre = pp.tile([128, NE_J, H], f32, tag="m")
                x2pre = pp.tile([128, NE_J, H], f32, tag="m")
                for j in range(NE_J):
                    lhsT = xeT_s[:, j * 128:(j + 1) * 128]
                    nc.tensor.matmul(x1pre[:, j, :], lhsT=lhsT, rhs=Wm1_s[:],
                                     start=True, stop=True)
                    nc.tensor.matmul(x2pre[:, j, :], lhsT=lhsT, rhs=Wm2_s[:],
                                     start=True, stop=True)

                def ln_relu(pre, g_s, be_s, dst_loc):
                    mu = wp.tile([128, NE_J], f32, tag="mu")
                    nc.vector.tensor_reduce(mu[:], pre[:, :, :],
                                            mybir.AxisListType.X, OP.add)
                    nc.vector.tensor_scalar_mul(mu[:], mu[:], 1.0 / H)
                    d = wp.tile([128, NE_J, H], f32, tag="d_ln")
                    nc.vector.tensor_tensor(
                        out=d[:, :, :], in0=pre[:, :, :],
                        in1=mu[:].unsqueeze(-1).broadcast_to([128, NE_J, H]),
                        op=OP.subtract)
                    d2 = wp.tile([128, NE_J, H], f32, tag="d2_ln")
                    nc.vector.tensor_tensor(out=d2[:, :, :], in0=d[:, :, :],
                                            in1=d[:, :, :], op=OP.mult)
                    vs = wp.tile([128, NE_J], f32, tag="vs")
                    nc.vector.tensor_reduce(vs[:], d2[:, :, :],
                                            mybir.AxisListType.X, OP.add)
                    nc.vector.tensor_scalar(out=vs[:], in0=vs[:],
                                            scalar1=1.0 / H, scalar2=LN_EPS,
                                            op0=OP.mult, op1=OP.add)
                    nc.scalar.activation(vs[:], vs[:], AF.Sqrt)
                    nc.vector.reciprocal(vs[:], vs[:])
                    y = wp.tile([128, NE_J, H], f32, tag="y_ln")
                    nc.vector.tensor_tensor(
                        out=y[:, :, :], in0=d[:, :, :],
                        in1=vs[:].unsqueeze(-1).broadcast_to([128, NE_J, H]),
                        op=OP.mult)
                    nc.vector.tensor_tensor(
                        out=y[:, :, :], in0=y[:, :, :],
                        in1=g_s[:].unsqueeze(1).broadcast_to([128, NE_J, H]),
                        op=OP.mult)
                    nc.vector.tensor_tensor(
                        out=y[:, :, :], in0=y[:, :, :],
                        in1=be_s[:].unsqueeze(1).broadcast_to([128, NE_J, H]),
                        op=OP.add)
                    x_out = wp.tile([128, NE_J, H], f32, tag="x_out")
                    nc.scalar.activation(x_out[:, :, :], y[:, :, :], AF.Relu)
                    # x_loc row (p*NE_J + j) = edge slot j*128+p
                    nc.sync.dma_start(
                        dst_loc[0:ne_pad, :].rearrange("(p b) h -> p (b h)", p=128),
                        x_out[:, :, :].rearrange("p b h -> p (b h)"))

                ln_relu(x1pre, gm1_s, bem1_s, x1_loc)
                ln_relu(x2pre, gm2_s, bem2_s, x2_loc)
                zt = wp.tile([8, H], f32, tag="zt")
                nc.vector.memset(zt[:], 0.0)
                nc.sync.dma_start(x1_loc[ne_pad:ne_pad + 8, :], zt[:])
                nc.sync.dma_start(x2_loc[ne_pad:ne_pad + 8, :], zt[:])

            if stages == 4:
                dmy = wp.tile([128, 1], f32, tag="dmy")
                nc.vector.memset(dmy[:], 1.0)
                nc.sync.dma_start(outq[:], dmy[:])

            if stages >= 5:
                # ---- pair contraction + combine ----
                ip1 = cp.tile([128, NI2 // 16], i16)
                ip2 = cp.tile([128, NI2 // 16], i16)
                nc.sync.dma_start(ip1[:], idxp1[:])
                nc.sync.dma_start(ip2[:], idxp2[:])
                g1 = cp.tile([128, M, H], f32)
                g2 = cp.tile([128, M, H], f32)
                gather_rows(g1, x2_loc[:], ip1, NI2)
                gather_rows(g2, x1_loc[:], ip2, NI2)
                prod = cp.tile([128, M, H], f32)
                nc.vector.tensor_tensor(out=prod[:, :, :], in0=g1[:, :, :],
                                        in1=g2[:, :, :], op=OP.mult)
                feat = cp.tile([128, 2 * H], f32)
                nc.vector.tensor_reduce(feat[:, 0:H],
                                        prod[:, :, :].transpose([0, 2, 1]),
                                        mybir.AxisListType.X, OP.add)
                nc.vector.tensor_tensor(out=feat[:, H:2 * H],
                                        in0=he0[:, NE_J, :],
                                        in1=he1[:, NE_J, :], op=OP.mult)

            if stages == 5:
                dmy = wp.tile([128, 1], f32, tag="dmy")
                nc.vector.tensor_copy(out=dmy[:], in_=feat[:, 0:1])
                nc.sync.dma_start(outq[:], dmy[:])

            if stages >= 6:
                # ---- final MLP ----
                ptf = pp.tile([128, 128], f32, tag="ps")
                nc.tensor.transpose(ptf[:], feat[:], identf[:])
                featT = cp.tile([128, 128], bf16)
                nc.scalar.activation(featT[:], ptf[:], AF.Copy)
                psh = pp.tile([128, H], f32, tag="ps")
                nc.tensor.matmul(psh[:], lhsT=featT[:], rhs=Wa_s[:],
                                 start=True, stop=True)
                hidf = wp.tile([128, H], f32, tag="hidf")
                nc.vector.tensor_tensor(out=hidf[:], in0=psh[:], in1=ba_s[:],
                                        op=OP.add)
                hid = wp.tile([128, H], bf16, tag="hid")
                nc.scalar.activation(hid[:], hidf[:], AF.Relu)
                pth = pp.tile([H, 128], bf16, tag="tpb")
                nc.tensor.transpose(pth[:], hid[:], identb[:])
                hidT = wp.tile([H, 128], bf16, tag="hidT")
                nc.scalar.activation(hidT[:], pth[:], AF.Copy)
                pso = pp.tile([128, 1], f32, tag="ps")
                nc.tensor.matmul(pso[:], lhsT=hidT[:], rhs=Wb_s[:],
                                 start=True, stop=True)
                ot = wp.tile([128, 1], f32, tag="ot")
                nc.vector.tensor_tensor(out=ot[:], in0=pso[:], in1=bb_s[:],
                                        op=OP.add)
                nc.sync.dma_start(outq[:], ot[:])

    nc.compile()
    return nc


def build_in_maps(inputs):
    """Returns (nc, in_maps) for the given full inputs."""
    import os
    inputs = {k: np.asarray(v) for k, v in inputs.items()}
    shared, cores = preprocess(inputs["ei"], inputs["pos"])
    M, NE_J = shared["M"], shared["NE_J"]
    stages = int(os.environ.get("K_STAGES", "6"))
    key = (M, NE_J, stages)
    if key not in _PROGRAM_CACHE:
        _PROGRAM_CACHE[key] = build_program(M, NE_J, stages)
    nc = _PROGRAM_CACHE[key]

    f = lambda k: np.ascontiguousarray(np.asarray(inputs[k], np.float32))
    b = lambda k: np.ascontiguousarray(np.asarray(inputs[k]).astype(BF16))
    rowf = lambda k: np.ascontiguousarray(
        np.broadcast_to(f(k).reshape(1, H), (128, H)))

    Wm1e = np.concatenate([f("W_m1"), f("b_m1").reshape(1, H)], 0).astype(BF16)
    Wm2e = np.concatenate([f("W_m2"), f("b_m2").reshape(1, H)], 0).astype(BF16)
    base = dict(
        xTb=np.ascontiguousarray(f("x").T.astype(BF16)),
        cntb=shared["cntb"], degf=shared["degf"],
        Wg1=b("W_g1"), Wg2=b("W_g2"),
        Wm1e=np.ascontiguousarray(Wm1e), Wm2e=np.ascontiguousarray(Wm2e),
        Wa=b("W_a"), Wb=b("W_b"),
        bg1r=rowf("b_g1"), bg2r=rowf("b_g2"),
        gm1r=rowf("g_m1"), bem1r=rowf("be_m1"),
        gm2r=rowf("g_m2"), bem2r=rowf("be_m2"),
        bar=rowf("b_a"),
        bbr=np.ascontiguousarray(
            np.broadcast_to(f("b_b").reshape(1, 1), (128, 1))),
    )
    in_maps = []
    for c in range(NCORES):
        m = dict(base)
        m.update(cores[c])
        in_maps.append(m)
    return nc, in_maps


def kernel(**inputs):
    from concourse.bass_utils import run_bass_kernel_spmd

    nc, in_maps = build_in_maps(inputs)
    try:
        res = run_bass_kernel_spmd(nc, in_maps, list(range(NCORES)))
        out = np.concatenate([res.results[c]["outq"].reshape(QPC)
                              for c in range(NCORES)])
    except Exception as e:
        # Hardware dispatch failed: execute the same compiled program
        # per-core in the cycle-accurate simulator.
        print("run_bass_kernel_spmd failed (%r); falling back to CoreSim" % (e,))
        from concourse.bass_interp import CoreSim
        outs = []
        for c in range(NCORES):
            sim = CoreSim(nc)
            for k, v in in_maps[c].items():
                sim.tensor(k)[:] = v
            sim.simulate(check_with_hw=False)
            outs.append(np.array(sim.tensor("outq")).reshape(QPC).copy())
        out = np.concatenate(outs)
    return out.astype(np.float32)


if __name__ == "__main__":
    import os
    os.environ.setdefault("JAX_PLATFORMS", "")
    import reference
    inputs = {k: np.asarray(v) for k, v in reference.setup_inputs().items()}
    got = kernel(**inputs)
    print(got[:8])
